# revision 30
# baseline (speedup 1.0000x reference)
"""Trainium2 Bass kernel for nn_CrossLayerRouter (MoE noisy-top-k router).

Reference computation (per token, D=2048, E=64, K=8):
    logits       = x @ Wr + br
    noise_logits = x @ Wn + bn
    noisy        = logits + noise_eps * softplus(noise_logits)
    topk, idx    = top_k(noisy, 8)
    router       = softmax(where(topk_mask, noisy, -inf))
    skip_prob    = sigmoid(x @ Ws + bs)

Distribution: data-parallel over tokens. B=8 batch rows x S=4096 tokens
= 32768 tokens; core i processes batch row i (4096 tokens). Router
weights are replicated. No collectives needed (disjoint outputs).

Device algorithm per core:
  1. Host folds Ws into the router weights:
         Wr_t = Wr - rowmean(Wr) + Ws
     Column-sum of the modified logits is then exactly 64*(x@Ws); the
     per-token shift (x@Ws - x@rowmean) cancels in both top-k and
     softmax, so router/indices are unchanged while the skip logit
     drops out of a column sum for free.
  2. Main matmul: stationary [Wr_t | Wn] (d-chunk x 128) streams xT
     (d-chunk x 512 tokens), accumulating psum [128=2E, 512]. x is
     pre-split on host into bf16 hi/lo (x = hi + lo to ~2^-18); three
     bf16 passes (hi@Whi, hi@Wlo, lo@Whi) give near-fp32 precision at
     1 cycle/row (fp32 native would be 4 cycles/row).
  3. Transpose matmul: stationary = 128x128 slice of the [2E, t] sbuf
     tile, moving = const [I_128 | m] (129 cols, m = ones on the 64
     logit rows). Output psum [128 tokens, 129]: cols 0:64 logits,
     64:128 noise logits, col 128 = logit column-sum = 64*skip_logit.
  4. Routing tail per 128-token chunk on DVE/ACT:
     softplus = Ln(1 + Exp(z)) (one ACT table set: natural_log_exp),
     noisy = logits + eps*softplus, DVE max/max_index for top-8 values
     + indices, softmax via Exp(bias=-max) with accum_out denominator,
     mask = (noisy >= t8) * (1/Z), sigmoid = 1/(1 + Exp(-s)).
"""

import os
import sys

import numpy as np

for _p in ("/opt/trn_rl_repo",):
    if _p not in sys.path:
        sys.path.insert(0, _p)

import ml_dtypes  # noqa: E402

import concourse.bass as bass  # noqa: E402
import concourse.mybir as mybir  # noqa: E402
import concourse.tile as tile  # noqa: E402
from concourse import bacc  # noqa: E402
from concourse import bass_utils  # noqa: E402

# Problem shapes (hardcoded per contest rules).
B, S, D, E, K = 8, 4096, 2048, 64, 8
NCORES = 8
T = (B * S) // NCORES  # tokens per core = 4096
P = 128
DC = D // P  # 16 contraction chunks
TT = 512  # token tile (psum free dim)
NTT = T // TT  # 8
NCH = T // P  # 32 chunks of 128 tokens
E2 = 2 * E  # 128
M_COLS = E2 + 1  # 129: [I_128 | logit-mask column]

F32 = mybir.dt.float32
BF16 = mybir.dt.bfloat16
U32 = mybir.dt.uint32

AF = mybir.ActivationFunctionType
OP = mybir.AluOpType

# "bf16x3": 3-pass bf16 hi/lo split (fast, ~6e-6 logit error)
# "f32":    native fp32 matmul (exact, 4 cycles/row)
MATMUL_MODE = os.environ.get("ROUTER_MATMUL_MODE", "bf16x3")


def build_nc(
    mode: str = MATMUL_MODE,
    bias_nonzero: bool = False,
    repeat: int = 1,
    loop_repeat: int = 1,
) -> bass.Bass:
    # Bacc, not raw Bass: its compile pipeline runs
    # move_matmul_waits_to_ldweights + generate_event_semaphores, which
    # legalize multi-semaphore waits (TRN2 allows 1 wait per instruction).
    nc = bacc.Bacc(None)

    if mode == "bf16x3":
        xpk = nc.dram_tensor("xpk", [P, NTT, 2, DC, TT], BF16, kind="ExternalInput")
        whi = nc.dram_tensor("whi", [P, DC, E2], BF16, kind="ExternalInput")
        wlo = nc.dram_tensor("wlo", [P, DC, E2], BF16, kind="ExternalInput")
    elif mode == "f32":
        x32 = nc.dram_tensor("x32", [P, NTT, DC, TT], F32, kind="ExternalInput")
        w32 = nc.dram_tensor("w32", [P, DC, E2], F32, kind="ExternalInput")
    else:
        raise ValueError(mode)

    eps = nc.dram_tensor("eps", [P, NCH, E], F32, kind="ExternalInput")
    mid = nc.dram_tensor("mid", [P, M_COLS], F32, kind="ExternalInput")
    skbias = nc.dram_tensor("skbias", [P, 1], F32, kind="ExternalInput")
    if bias_nonzero:
        # br||bn replicated across partitions (token rows)
        biasrep = nc.dram_tensor("biasrep", [P, E2], F32, kind="ExternalInput")

    router = nc.dram_tensor("router", [P, NCH, E], F32, kind="ExternalOutput")
    idx = nc.dram_tensor("idx", [P, NCH, K], U32, kind="ExternalOutput")
    skp = nc.dram_tensor("skp", [P, NCH], F32, kind="ExternalOutput")

    with (
        tile.TileContext(nc) as tc,
        tc.tile_pool(name="consts", bufs=1) as consts,
        tc.tile_pool(name="xin", bufs=4) as xin,
        tc.tile_pool(name="lnp", bufs=3) as lnp,
        tc.tile_pool(name="work", bufs=6) as work,
        tc.tile_pool(name="outacc", bufs=1) as outacc,
        tc.tile_pool(name="psmm", bufs=3, space="PSUM") as psmm,
        tc.tile_pool(name="pstr", bufs=4, space="PSUM") as pstr,
    ):
        # ---- constants ----
        if mode == "bf16x3":
            whi_sb = consts.tile([P, DC, E2], BF16)
            nc.sync.dma_start(whi_sb[:], whi[:])
            wlo_sb = consts.tile([P, DC, E2], BF16)
            nc.sync.dma_start(wlo_sb[:], wlo[:])
        else:
            w32_sb = consts.tile([P, DC, E2], F32)
            nc.sync.dma_start(w32_sb[:], w32[:])
        mid_sb = consts.tile([P, M_COLS], F32)
        nc.sync.dma_start(mid_sb[:], mid[:])
        skb_sb = consts.tile([P, 1], F32)
        nc.sync.dma_start(skb_sb[:], skbias[:])
        if bias_nonzero:
            biasrep_sb = consts.tile([P, E2], F32)
            nc.sync.dma_start(biasrep_sb[:], biasrep[:])
        eps_sb = consts.tile([P, NCH, E], F32)
        nc.sync.dma_start(eps_sb[:], eps[:])

        # ---- output accumulators (one DMA each at the end) ----
        router_sb = outacc.tile([P, NCH, E], F32)
        idx_sb = outacc.tile([P, NCH, K], U32)
        skp_sb = outacc.tile([P, NCH], F32)

        # Prime PE's vector clock on each PE-read constant with a dummy
        # matmul carrying exactly one DMA wait apiece: the LDWEIGHTS
        # struct has a single sync-wait slot, so real matmuls must never
        # combine a const-DMA wait with a data wait.
        with tc.tile_pool(name="prime", bufs=1, space="PSUM") as prime:
            dummy_ps = prime.tile([P, P], F32)
            if mode == "bf16x3":
                nc.tensor.matmul(
                    dummy_ps[:], lhsT=whi_sb[:, 0], rhs=whi_sb[:, 0],
                    start=True, stop=True,
                )
                nc.tensor.matmul(
                    dummy_ps[:], lhsT=wlo_sb[:, 0], rhs=wlo_sb[:, 0],
                    start=True, stop=True,
                )
            else:
                nc.tensor.matmul(
                    dummy_ps[:], lhsT=w32_sb[:, 0], rhs=w32_sb[:, 0],
                    start=True, stop=True,
                )
            nc.tensor.matmul(
                dummy_ps[:], lhsT=mid_sb[:, 0:P], rhs=mid_sb[:, 0:P],
                start=True, stop=True,
            )

        # Prime DVE on eps and ACT on skbias the same way (single-wait
        # dummies), so steady-state ops never pair a const-DMA wait with
        # a compute wait.
        with tc.tile_pool(name="prime_sb", bufs=1) as prime_sb:
            scr_d = prime_sb.tile([P, 8], F32)
            nc.vector.tensor_copy(scr_d[:], eps_sb[:, 0, 0:8])
            scr_a = prime_sb.tile([P, 1], F32)
            nc.scalar.activation(scr_a[:], skb_sb[:], AF.Copy)
            if bias_nonzero:
                scr_b = prime_sb.tile([P, 8], F32)
                nc.vector.tensor_copy(scr_b[:], biasrep_sb[:, 0:8])

        def emit_body():
            for tt_rep in range(repeat * NTT):
                emit_tile(tt_rep % NTT)

        def emit_tile(tt):
            # stream this token tile's x columns
            if mode == "bf16x3":
                xpk_t = xin.tile([P, 2, DC, TT], BF16, tag="xpk")
                nc.sync.dma_start(xpk_t[:], xpk[:, tt])
                xhi_t = xpk_t[:, 0]
                xlo_t = xpk_t[:, 1]
                passes = [(whi_sb, xhi_t), (wlo_sb, xhi_t), (whi_sb, xlo_t)]
            else:
                x32_t = xin.tile([P, DC, TT], F32, tag="x32")
                nc.sync.dma_start(x32_t[:], x32[:, tt])
                passes = [(w32_sb, x32_t)]

            ps = psmm.tile([P, TT], F32)
            n_mm = len(passes) * DC
            i = 0
            for wt, xt in passes:
                for c in range(DC):
                    nc.tensor.matmul(
                        ps[:],
                        lhsT=wt[:, c],
                        rhs=xt[:, c],
                        start=(i == 0),
                        stop=(i == n_mm - 1),
                    )
                    i += 1

            # psum -> sbuf with per-partition bias (br||bn) fused in
            lnt = lnp.tile([P, TT], F32, tag="lnt")
            nc.vector.tensor_copy(lnt[:], ps[:])

            for q in range(TT // P):
                ch = tt * (TT // P) + q
                # transpose 128 tokens + logit column-sum (col 128)
                pst = pstr.tile([P, M_COLS], F32)
                nc.tensor.matmul(
                    pst[:],
                    lhsT=lnt[:, q * P : (q + 1) * P],
                    rhs=mid_sb[:],
                    start=True,
                    stop=True,
                )

                # Single DVE copy is the only psum consumer, so the next
                # transpose matmul's WAR dep collapses to one engine
                # (walrus allows at most 2 sync waits per matmul).
                pstc = work.tile([P, M_COLS], F32, tag="pstc")
                nc.vector.tensor_copy(pstc[:], pst[:])
                if bias_nonzero:
                    nc.vector.tensor_add(
                        pstc[:, 0:E2], pstc[:, 0:E2], biasrep_sb[:]
                    )

                # softplus(noise) = Ln(1 + Exp(z))
                expn = work.tile([P, E], F32, tag="expn")
                nc.scalar.activation(expn[:], pstc[:, E:E2], AF.Exp)
                sp = work.tile([P, E], F32, tag="sp")
                nc.scalar.activation(sp[:], expn[:], AF.Ln, bias=1.0)

                # noisy = logits + eps * softplus
                tmp = work.tile([P, E], F32, tag="tmp")
                nc.vector.tensor_mul(tmp[:], eps_sb[:, ch], sp[:])
                noisy = work.tile([P, E], F32, tag="noisy")
                nc.vector.tensor_add(noisy[:], pstc[:, 0:E], tmp[:])

                # top-8 values + indices
                m8 = work.tile([P, 8], F32, tag="m8")
                nc.vector.max(out=m8[:], in_=noisy[:])
                nc.vector.max_index(
                    out=idx_sb[:, ch], in_max=m8[:], in_values=noisy[:]
                )

                # Masked softmax, restructured so every instruction carries
                # at most ONE semaphore dependency (walrus 1-wait limit):
                #   router = exp(noisy_masked - m1 - ln(Z))
                # with Z = sum of top-8 exp(m8 - m1), masked logits pushed
                # to -1e30 so exp underflows to exactly 0.  The final exp
                # runs on ACT and writes router_sb directly.
                negm = work.tile([P, 1], F32, tag="negm")
                nc.vector.tensor_scalar(negm[:], m8[:, 0:1], -1.0, None, OP.mult)
                z8 = work.tile([P, 8], F32, tag="z8")
                zsum = work.tile([P, 1], F32, tag="zsum")
                nc.scalar.activation(
                    z8[:], m8[:], AF.Exp, bias=negm[:], accum_out=zsum[:]
                )
                lnz = work.tile([P, 1], F32, tag="lnz")
                nc.scalar.activation(lnz[:], zsum[:], AF.Ln)
                # nb2 = -(m1 + lnZ) = negm - lnz
                nb2 = work.tile([P, 1], F32, tag="nb2")
                nc.vector.tensor_tensor(nb2[:], negm[:], lnz[:], OP.subtract)
                # mask: (noisy < t8) * -1e30, added to noisy
                mneg = work.tile([P, E], F32, tag="mneg")
                nc.vector.tensor_scalar(
                    mneg[:], noisy[:], m8[:, 7:8], -1.0e30, OP.is_lt, OP.mult
                )
                noisy_m = work.tile([P, E], F32, tag="noisy_m")
                nc.vector.tensor_add(noisy_m[:], noisy[:], mneg[:])
                nc.scalar.activation(
                    router_sb[:, ch], noisy_m[:], AF.Exp, bias=nb2[:]
                )

                # skip_prob = 1 / (1 + exp(-S1/64 - bs))
                esk = work.tile([P, 1], F32, tag="esk")
                nc.scalar.activation(
                    esk[:], pstc[:, E2 : E2 + 1], AF.Exp,
                    bias=skb_sb[:], scale=-1.0 / 64.0,
                )
                esk1 = work.tile([P, 1], F32, tag="esk1")
                nc.vector.tensor_scalar(esk1[:], esk[:], 1.0, None, OP.add)
                nc.vector.reciprocal(skp_sb[:, ch : ch + 1], esk1[:])

        if loop_repeat > 1:
            # Timing-only variant: run the whole body loop_repeat times in
            # one dispatch (results are idempotent).
            with tc.For_i(0, loop_repeat, 1):
                emit_body()
        else:
            emit_body()

        nc.sync.dma_start(router[:], router_sb[:])
        nc.sync.dma_start(idx[:], idx_sb[:])
        nc.sync.dma_start(skp[:], skp_sb[:])

    # Bacc's compile pipeline (wait legalization, register allocation) runs
    # in finalize(); the axon/PJRT execution path does not call it for us.
    #
    # The ACT table-set picker chooses per-function greedily, alternating
    # exp-only / ln-only sets every chunk (~2.7us per reload, ~170us/kernel).
    # All our activations (Exp, Ln, Copy) live together in
    # natural_log_exp_and_others; blank out every set that lacks one of
    # them (positions preserved -- act_func_set_id is positional) so the
    # picker emits exactly one table load.
    import concourse.hw_specs as hw_specs

    needed = {AF.Exp, AF.Ln, AF.Copy}
    real_tables = hw_specs.get_activation_tables(nc.m.arch)
    patched = {
        name: (fns if needed <= fns else set())
        for name, fns in real_tables.items()
    }
    assert any(patched.values()), "no ACT set contains Exp+Ln+Copy"
    orig = bacc.get_activation_tables
    bacc.get_activation_tables = lambda arch: patched
    try:
        nc.finalize()
    finally:
        bacc.get_activation_tables = orig
    return nc


def _rearrange_xT(xT: np.ndarray) -> np.ndarray:
    """[D, T] -> [P, NTT, DC, TT] with x_r[p, tt, c, t] = xT[c*128+p, tt*512+t]."""
    return np.ascontiguousarray(
        xT.reshape(DC, P, NTT, TT).transpose(1, 2, 0, 3)
    )


def _prep_host(x, noise_eps, Wr, br, Wn, bn, Ws, bs, mode: str):
    """Build the per-core input maps."""
    x = np.asarray(x, dtype=np.float32)
    noise_eps = np.asarray(noise_eps, dtype=np.float32)
    Wr = np.asarray(Wr, dtype=np.float32)
    br = np.asarray(br, dtype=np.float32)
    Wn = np.asarray(Wn, dtype=np.float32)
    bn = np.asarray(bn, dtype=np.float32)
    Ws = np.asarray(Ws, dtype=np.float32)
    bs = np.asarray(bs, dtype=np.float32)

    # Fold Ws into Wr (float64 so the column-sum identity holds to fp32)
    Wr64 = Wr.astype(np.float64)
    Wr_t = Wr64 - Wr64.mean(axis=1, keepdims=True) + Ws.astype(np.float64)
    W_comb = np.concatenate([Wr_t, Wn.astype(np.float64)], axis=1)  # [D, 128]

    def w_rearrange(w):
        # [D, E2] -> [P, DC, E2]
        return np.ascontiguousarray(w.reshape(DC, P, E2).transpose(1, 0, 2))

    common = {}
    W32 = W_comb.astype(np.float32)
    if mode == "bf16x3":
        whi = W32.astype(ml_dtypes.bfloat16)
        wlo = (W32 - whi.astype(np.float32)).astype(ml_dtypes.bfloat16)
        common["whi"] = w_rearrange(whi)
        common["wlo"] = w_rearrange(wlo)
    else:
        common["w32"] = w_rearrange(W32)

    # [I_128 | mask column (1 on logit rows)]
    mid = np.zeros((P, M_COLS), dtype=np.float32)
    mid[:, :E2] = np.eye(P, dtype=np.float32)
    mid[:E, E2] = 1.0
    common["mid"] = mid

    bias_nonzero = bool(br.any() or bn.any())
    if bias_nonzero:
        common["biasrep"] = np.ascontiguousarray(
            np.tile(np.concatenate([br, bn]).astype(np.float32), (P, 1))
        )

    # exp(-z), z = S1/64 + bs  ->  Exp(S1*(-1/64) + (-bs))
    skb = np.full((P, 1), -float(bs[0]), dtype=np.float32)
    common["skbias"] = skb

    common["__bias_nonzero__"] = bias_nonzero
    in_maps = []
    for core in range(NCORES):
        xi = x[core]  # [S, D] tokens for this core (batch row)
        xT = np.ascontiguousarray(xi.T)  # [D, T]
        m = dict(common)
        if mode == "bf16x3":
            xhi = xT.astype(ml_dtypes.bfloat16)
            xlo = (xT - xhi.astype(np.float32)).astype(ml_dtypes.bfloat16)
            m["xpk"] = np.ascontiguousarray(
                np.stack([_rearrange_xT(xhi), _rearrange_xT(xlo)], axis=2)
            )
        else:
            m["x32"] = _rearrange_xT(xT)
        ei = noise_eps[core]  # [S, E]
        m["eps"] = np.ascontiguousarray(
            ei.reshape(NCH, P, E).transpose(1, 0, 2)
        )
        in_maps.append(m)
    return in_maps


def _postprocess(results):
    routers, idxs, skips = [], [], []
    for r in results:
        # [P, NCH, ...] with token = ch*128 + p  -> [T, ...]
        routers.append(
            np.ascontiguousarray(r["router"].transpose(1, 0, 2)).reshape(T, E)
        )
        idxs.append(
            np.ascontiguousarray(r["idx"].transpose(1, 0, 2))
            .reshape(T, K)
            .astype(np.int32)
        )
        skips.append(np.ascontiguousarray(r["skp"].T).reshape(T, 1))
    router = np.stack(routers).reshape(B, S, E).astype(np.float32)
    indices = np.stack(idxs).reshape(B, S, K).astype(np.int32)
    skip = np.stack(skips).reshape(B, S, 1).astype(np.float32)
    return router, indices, skip


_NC_CACHE: dict[tuple, bass.Bass] = {}


def _get_nc(mode: str, bias_nonzero: bool) -> bass.Bass:
    key = (mode, bias_nonzero)
    if key not in _NC_CACHE:
        _NC_CACHE[key] = build_nc(mode, bias_nonzero)
    return _NC_CACHE[key]


def run(inputs: dict, mode: str = MATMUL_MODE, **spmd_kwargs):
    """Run on hardware; returns ((router, indices, skip), BassKernelResults)."""
    in_maps = _prep_host(mode=mode, **inputs)
    bias_nonzero = in_maps[0].pop("__bias_nonzero__")
    for m in in_maps[1:]:
        m.pop("__bias_nonzero__")
    nc = _get_nc(mode, bias_nonzero)
    res = bass_utils.run_bass_kernel_spmd(
        nc, in_maps, core_ids=list(range(NCORES)), **spmd_kwargs
    )
    return _postprocess(res.results), res


def kernel(**inputs) -> tuple:
    (router, indices, skip), _ = run(inputs)
    return router, indices, skip


# revision 32
# speedup vs baseline: 17.4000x; 17.4000x over previous
"""Trainium2 Bass kernel for nn_CrossLayerRouter (MoE noisy-top-k router).

Reference computation (per token, D=2048, E=64, K=8):
    logits       = x @ Wr + br
    noise_logits = x @ Wn + bn
    noisy        = logits + noise_eps * softplus(noise_logits)
    topk, idx    = top_k(noisy, 8)
    router       = softmax(where(topk_mask, noisy, -inf))
    skip_prob    = sigmoid(x @ Ws + bs)

Distribution: data-parallel over tokens. B=8 batch rows x S=4096 tokens
= 32768 tokens; core i processes batch row i (4096 tokens). Router
weights are replicated. No collectives needed (disjoint outputs).

Device algorithm per core:
  1. Host folds Ws into the router weights:
         Wr_t = Wr - rowmean(Wr) + Ws
     Column-sum of the modified logits is then exactly 64*(x@Ws); the
     per-token shift (x@Ws - x@rowmean) cancels in both top-k and
     softmax, so router/indices are unchanged while the skip logit
     drops out of a column sum for free.
  2. Main matmul: stationary [Wr_t | Wn] (d-chunk x 128) streams xT
     (d-chunk x 512 tokens), accumulating psum [128=2E, 512]. x is
     pre-split on host into bf16 hi/lo (x = hi + lo to ~2^-18); three
     bf16 passes (hi@Whi, hi@Wlo, lo@Whi) give near-fp32 precision at
     1 cycle/row (fp32 native would be 4 cycles/row).
  3. Transpose matmul: stationary = 128x128 slice of the [2E, t] sbuf
     tile, moving = const [I_128 | m] (129 cols, m = ones on the 64
     logit rows). Output psum [128 tokens, 129]: cols 0:64 logits,
     64:128 noise logits, col 128 = logit column-sum = 64*skip_logit.
  4. Routing tail per 128-token chunk on DVE/ACT:
     softplus = Ln(1 + Exp(z)) (one ACT table set: natural_log_exp),
     noisy = logits + eps*softplus, DVE max/max_index for top-8 values
     + indices, softmax via Exp(bias=-max) with accum_out denominator,
     mask = (noisy >= t8) * (1/Z), sigmoid = 1/(1 + Exp(-s)).
"""

import os
import sys

import numpy as np

for _p in ("/opt/trn_rl_repo",):
    if _p not in sys.path:
        sys.path.insert(0, _p)

import ml_dtypes  # noqa: E402

import concourse.bass as bass  # noqa: E402
import concourse.mybir as mybir  # noqa: E402
import concourse.tile as tile  # noqa: E402
from concourse import bacc  # noqa: E402
from concourse import bass_utils  # noqa: E402

# Problem shapes (hardcoded per contest rules).
B, S, D, E, K = 8, 4096, 2048, 64, 8
NCORES = 8
T = (B * S) // NCORES  # tokens per core = 4096
P = 128
DC = D // P  # 16 contraction chunks
TT = 512  # token tile (psum free dim)
NTT = T // TT  # 8
NCH = T // P  # 32 chunks of 128 tokens
E2 = 2 * E  # 128
M_COLS = E2 + 1  # 129: [I_128 | logit-mask column]

F32 = mybir.dt.float32
BF16 = mybir.dt.bfloat16
U32 = mybir.dt.uint32

AF = mybir.ActivationFunctionType
OP = mybir.AluOpType

# "bf16x3": 3-pass bf16 hi/lo split (fast, ~6e-6 logit error)
# "f32":    native fp32 matmul (exact, 4 cycles/row)
MATMUL_MODE = os.environ.get("ROUTER_MATMUL_MODE", "bf16x3")


def build_nc(
    mode: str = MATMUL_MODE,
    bias_nonzero: bool = False,
    repeat: int = 1,
    loop_repeat: int = 1,
) -> bass.Bass:
    # Bacc, not raw Bass: its compile pipeline runs
    # move_matmul_waits_to_ldweights + generate_event_semaphores, which
    # legalize multi-semaphore waits (TRN2 allows 1 wait per instruction).
    nc = bacc.Bacc(None)

    if mode == "bf16x3":
        xhi = nc.dram_tensor("xhi", [P, NTT, DC, TT], BF16, kind="ExternalInput")
        xlo = nc.dram_tensor("xlo", [P, NTT, DC, TT], BF16, kind="ExternalInput")
        whi = nc.dram_tensor("whi", [P, DC, E2], BF16, kind="ExternalInput")
        wlo = nc.dram_tensor("wlo", [P, DC, E2], BF16, kind="ExternalInput")
    elif mode == "f32":
        x32 = nc.dram_tensor("x32", [P, NTT, DC, TT], F32, kind="ExternalInput")
        w32 = nc.dram_tensor("w32", [P, DC, E2], F32, kind="ExternalInput")
    else:
        raise ValueError(mode)

    eps = nc.dram_tensor("eps", [P, NCH, E], F32, kind="ExternalInput")
    mid = nc.dram_tensor("mid", [P, M_COLS], F32, kind="ExternalInput")
    skbias = nc.dram_tensor("skbias", [P, 1], F32, kind="ExternalInput")
    if bias_nonzero:
        # br||bn replicated across partitions (token rows)
        biasrep = nc.dram_tensor("biasrep", [P, E2], F32, kind="ExternalInput")

    router = nc.dram_tensor("router", [P, NCH, E], F32, kind="ExternalOutput")
    idx = nc.dram_tensor("idx", [P, NCH, K], U32, kind="ExternalOutput")
    skp = nc.dram_tensor("skp", [P, NCH], F32, kind="ExternalOutput")

    with (
        tile.TileContext(nc) as tc,
        tc.tile_pool(name="consts", bufs=1) as consts,
        tc.tile_pool(name="xin", bufs=4) as xin,
        tc.tile_pool(name="lnp", bufs=3) as lnp,
        tc.tile_pool(name="work", bufs=6) as work,
        tc.tile_pool(name="outacc", bufs=1) as outacc,
        tc.tile_pool(name="psmm", bufs=3, space="PSUM") as psmm,
        tc.tile_pool(name="pstr", bufs=4, space="PSUM") as pstr,
    ):
        # ---- constants ----
        if mode == "bf16x3":
            whi_sb = consts.tile([P, DC, E2], BF16)
            nc.sync.dma_start(whi_sb[:], whi[:])
            wlo_sb = consts.tile([P, DC, E2], BF16)
            nc.sync.dma_start(wlo_sb[:], wlo[:])
        else:
            w32_sb = consts.tile([P, DC, E2], F32)
            nc.sync.dma_start(w32_sb[:], w32[:])
        mid_sb = consts.tile([P, M_COLS], F32)
        nc.sync.dma_start(mid_sb[:], mid[:])
        skb_sb = consts.tile([P, 1], F32)
        nc.sync.dma_start(skb_sb[:], skbias[:])
        if bias_nonzero:
            biasrep_sb = consts.tile([P, E2], F32)
            nc.sync.dma_start(biasrep_sb[:], biasrep[:])
        eps_sb = consts.tile([P, NCH, E], F32)
        nc.sync.dma_start(eps_sb[:], eps[:])

        # ---- output accumulators (one DMA each at the end) ----
        router_sb = outacc.tile([P, NCH, E], F32)
        idx_sb = outacc.tile([P, NCH, K], U32)
        skp_sb = outacc.tile([P, NCH], F32)

        # Prime PE's vector clock on each PE-read constant with a dummy
        # matmul carrying exactly one DMA wait apiece: the LDWEIGHTS
        # struct has a single sync-wait slot, so real matmuls must never
        # combine a const-DMA wait with a data wait.
        with tc.tile_pool(name="prime", bufs=1, space="PSUM") as prime:
            dummy_ps = prime.tile([P, P], F32)
            if mode == "bf16x3":
                nc.tensor.matmul(
                    dummy_ps[:], lhsT=whi_sb[:, 0], rhs=whi_sb[:, 0],
                    start=True, stop=True,
                )
                nc.tensor.matmul(
                    dummy_ps[:], lhsT=wlo_sb[:, 0], rhs=wlo_sb[:, 0],
                    start=True, stop=True,
                )
            else:
                nc.tensor.matmul(
                    dummy_ps[:], lhsT=w32_sb[:, 0], rhs=w32_sb[:, 0],
                    start=True, stop=True,
                )
            nc.tensor.matmul(
                dummy_ps[:], lhsT=mid_sb[:, 0:P], rhs=mid_sb[:, 0:P],
                start=True, stop=True,
            )

        # Prime DVE on eps and ACT on skbias the same way (single-wait
        # dummies), so steady-state ops never pair a const-DMA wait with
        # a compute wait.
        with tc.tile_pool(name="prime_sb", bufs=1) as prime_sb:
            scr_d = prime_sb.tile([P, 8], F32)
            nc.vector.tensor_copy(scr_d[:], eps_sb[:, 0, 0:8])
            scr_a = prime_sb.tile([P, 1], F32)
            nc.scalar.activation(scr_a[:], skb_sb[:], AF.Copy)
            if bias_nonzero:
                scr_b = prime_sb.tile([P, 8], F32)
                nc.vector.tensor_copy(scr_b[:], biasrep_sb[:, 0:8])

        def emit_body():
            for tt_rep in range(repeat * NTT):
                emit_tile(tt_rep % NTT)

        def emit_tile(tt):
            # stream this token tile's x columns
            if mode == "bf16x3":
                # 1MB halves: finer arrival granularity lets the first
                # matmuls start at 1MB, and two in-flight transfers per
                # tensor engage more DMA queue rows concurrently.
                xhi_t = xin.tile([P, DC, TT], BF16, tag="xhi")
                nc.sync.dma_start(xhi_t[:, 0 : DC // 2], xhi[:, tt, 0 : DC // 2])
                nc.sync.dma_start(xhi_t[:, DC // 2 :], xhi[:, tt, DC // 2 :])
                xlo_t = xin.tile([P, DC, TT], BF16, tag="xlo")
                nc.sync.dma_start(xlo_t[:, 0 : DC // 2], xlo[:, tt, 0 : DC // 2])
                nc.sync.dma_start(xlo_t[:, DC // 2 :], xlo[:, tt, DC // 2 :])
                passes = [(whi_sb, xhi_t), (wlo_sb, xhi_t), (whi_sb, xlo_t)]
            else:
                x32_t = xin.tile([P, DC, TT], F32, tag="x32")
                nc.sync.dma_start(x32_t[:], x32[:, tt])
                passes = [(w32_sb, x32_t)]

            ps = psmm.tile([P, TT], F32)
            n_mm = len(passes) * DC
            i = 0
            for wt, xt in passes:
                for c in range(DC):
                    nc.tensor.matmul(
                        ps[:],
                        lhsT=wt[:, c],
                        rhs=xt[:, c],
                        start=(i == 0),
                        stop=(i == n_mm - 1),
                    )
                    i += 1

            # psum -> sbuf with per-partition bias (br||bn) fused in
            lnt = lnp.tile([P, TT], F32, tag="lnt")
            nc.vector.tensor_copy(lnt[:], ps[:])

            for q in range(TT // P):
                ch = tt * (TT // P) + q
                # transpose 128 tokens + logit column-sum (col 128)
                pst = pstr.tile([P, M_COLS], F32)
                nc.tensor.matmul(
                    pst[:],
                    lhsT=lnt[:, q * P : (q + 1) * P],
                    rhs=mid_sb[:],
                    start=True,
                    stop=True,
                )

                # Single DVE copy is the only psum consumer, so the next
                # transpose matmul's WAR dep collapses to one engine
                # (walrus allows at most 2 sync waits per matmul).
                pstc = work.tile([P, M_COLS], F32, tag="pstc")
                nc.vector.tensor_copy(pstc[:], pst[:])
                if bias_nonzero:
                    nc.vector.tensor_add(
                        pstc[:, 0:E2], pstc[:, 0:E2], biasrep_sb[:]
                    )

                # softplus(noise) = Ln(1 + Exp(z))
                expn = work.tile([P, E], F32, tag="expn")
                nc.scalar.activation(expn[:], pstc[:, E:E2], AF.Exp)
                sp = work.tile([P, E], F32, tag="sp")
                nc.scalar.activation(sp[:], expn[:], AF.Ln, bias=1.0)

                # noisy = logits + eps * softplus
                tmp = work.tile([P, E], F32, tag="tmp")
                nc.vector.tensor_mul(tmp[:], eps_sb[:, ch], sp[:])
                noisy = work.tile([P, E], F32, tag="noisy")
                nc.vector.tensor_add(noisy[:], pstc[:, 0:E], tmp[:])

                # top-8 values + indices
                m8 = work.tile([P, 8], F32, tag="m8")
                nc.vector.max(out=m8[:], in_=noisy[:])
                nc.vector.max_index(
                    out=idx_sb[:, ch], in_max=m8[:], in_values=noisy[:]
                )

                # Masked softmax, restructured so every instruction carries
                # at most ONE semaphore dependency (walrus 1-wait limit):
                #   router = exp(noisy_masked - m1 - ln(Z))
                # with Z = sum of top-8 exp(m8 - m1), masked logits pushed
                # to -1e30 so exp underflows to exactly 0.  The final exp
                # runs on ACT and writes router_sb directly.
                negm = work.tile([P, 1], F32, tag="negm")
                nc.vector.tensor_scalar(negm[:], m8[:, 0:1], -1.0, None, OP.mult)
                z8 = work.tile([P, 8], F32, tag="z8")
                zsum = work.tile([P, 1], F32, tag="zsum")
                nc.scalar.activation(
                    z8[:], m8[:], AF.Exp, bias=negm[:], accum_out=zsum[:]
                )
                lnz = work.tile([P, 1], F32, tag="lnz")
                nc.scalar.activation(lnz[:], zsum[:], AF.Ln)
                # nb2 = -(m1 + lnZ) = negm - lnz
                nb2 = work.tile([P, 1], F32, tag="nb2")
                nc.vector.tensor_tensor(nb2[:], negm[:], lnz[:], OP.subtract)
                # mask: (noisy < t8) * -1e30, added to noisy
                mneg = work.tile([P, E], F32, tag="mneg")
                nc.vector.tensor_scalar(
                    mneg[:], noisy[:], m8[:, 7:8], -1.0e30, OP.is_lt, OP.mult
                )
                noisy_m = work.tile([P, E], F32, tag="noisy_m")
                nc.vector.tensor_add(noisy_m[:], noisy[:], mneg[:])
                nc.scalar.activation(
                    router_sb[:, ch], noisy_m[:], AF.Exp, bias=nb2[:]
                )

                # skip_prob = 1 / (1 + exp(-S1/64 - bs))
                esk = work.tile([P, 1], F32, tag="esk")
                nc.scalar.activation(
                    esk[:], pstc[:, E2 : E2 + 1], AF.Exp,
                    bias=skb_sb[:], scale=-1.0 / 64.0,
                )
                esk1 = work.tile([P, 1], F32, tag="esk1")
                nc.vector.tensor_scalar(esk1[:], esk[:], 1.0, None, OP.add)
                nc.vector.reciprocal(skp_sb[:, ch : ch + 1], esk1[:])

            # stream this tile's router slice out now (keeps the kernel
            # tail to just the small idx/skp transfers)
            nc.sync.dma_start(
                router[:, tt * 4 : (tt + 1) * 4], router_sb[:, tt * 4 : (tt + 1) * 4]
            )

        if loop_repeat > 1:
            # Timing-only variant: run the whole body loop_repeat times in
            # one dispatch (results are idempotent).
            with tc.For_i(0, loop_repeat, 1):
                emit_body()
        else:
            emit_body()

        nc.sync.dma_start(idx[:], idx_sb[:])
        nc.sync.dma_start(skp[:], skp_sb[:])

    # Bacc's compile pipeline (wait legalization, register allocation) runs
    # in finalize(); the axon/PJRT execution path does not call it for us.
    #
    # The ACT table-set picker chooses per-function greedily, alternating
    # exp-only / ln-only sets every chunk (~2.7us per reload, ~170us/kernel).
    # All our activations (Exp, Ln, Copy) live together in
    # natural_log_exp_and_others; blank out every set that lacks one of
    # them (positions preserved -- act_func_set_id is positional) so the
    # picker emits exactly one table load.
    import concourse.hw_specs as hw_specs

    needed = {AF.Exp, AF.Ln, AF.Copy}
    real_tables = hw_specs.get_activation_tables(nc.m.arch)
    patched = {
        name: (fns if needed <= fns else set())
        for name, fns in real_tables.items()
    }
    assert any(patched.values()), "no ACT set contains Exp+Ln+Copy"
    orig = bacc.get_activation_tables
    bacc.get_activation_tables = lambda arch: patched
    try:
        nc.finalize()
    finally:
        bacc.get_activation_tables = orig
    return nc


def _rearrange_xT(xT: np.ndarray) -> np.ndarray:
    """[D, T] -> [P, NTT, DC, TT] with x_r[p, tt, c, t] = xT[c*128+p, tt*512+t]."""
    return np.ascontiguousarray(
        xT.reshape(DC, P, NTT, TT).transpose(1, 2, 0, 3)
    )


def _prep_host(x, noise_eps, Wr, br, Wn, bn, Ws, bs, mode: str):
    """Build the per-core input maps."""
    x = np.asarray(x, dtype=np.float32)
    noise_eps = np.asarray(noise_eps, dtype=np.float32)
    Wr = np.asarray(Wr, dtype=np.float32)
    br = np.asarray(br, dtype=np.float32)
    Wn = np.asarray(Wn, dtype=np.float32)
    bn = np.asarray(bn, dtype=np.float32)
    Ws = np.asarray(Ws, dtype=np.float32)
    bs = np.asarray(bs, dtype=np.float32)

    # Fold Ws into Wr (float64 so the column-sum identity holds to fp32)
    Wr64 = Wr.astype(np.float64)
    Wr_t = Wr64 - Wr64.mean(axis=1, keepdims=True) + Ws.astype(np.float64)
    W_comb = np.concatenate([Wr_t, Wn.astype(np.float64)], axis=1)  # [D, 128]

    def w_rearrange(w):
        # [D, E2] -> [P, DC, E2]
        return np.ascontiguousarray(w.reshape(DC, P, E2).transpose(1, 0, 2))

    common = {}
    W32 = W_comb.astype(np.float32)
    if mode == "bf16x3":
        whi = W32.astype(ml_dtypes.bfloat16)
        wlo = (W32 - whi.astype(np.float32)).astype(ml_dtypes.bfloat16)
        common["whi"] = w_rearrange(whi)
        common["wlo"] = w_rearrange(wlo)
    else:
        common["w32"] = w_rearrange(W32)

    # [I_128 | mask column (1 on logit rows)]
    mid = np.zeros((P, M_COLS), dtype=np.float32)
    mid[:, :E2] = np.eye(P, dtype=np.float32)
    mid[:E, E2] = 1.0
    common["mid"] = mid

    bias_nonzero = bool(br.any() or bn.any())
    if bias_nonzero:
        common["biasrep"] = np.ascontiguousarray(
            np.tile(np.concatenate([br, bn]).astype(np.float32), (P, 1))
        )

    # exp(-z), z = S1/64 + bs  ->  Exp(S1*(-1/64) + (-bs))
    skb = np.full((P, 1), -float(bs[0]), dtype=np.float32)
    common["skbias"] = skb

    common["__bias_nonzero__"] = bias_nonzero
    in_maps = []
    for core in range(NCORES):
        xi = x[core]  # [S, D] tokens for this core (batch row)
        xT = np.ascontiguousarray(xi.T)  # [D, T]
        m = dict(common)
        if mode == "bf16x3":
            xhi = xT.astype(ml_dtypes.bfloat16)
            xlo = (xT - xhi.astype(np.float32)).astype(ml_dtypes.bfloat16)
            m["xhi"] = _rearrange_xT(xhi)
            m["xlo"] = _rearrange_xT(xlo)
        else:
            m["x32"] = _rearrange_xT(xT)
        ei = noise_eps[core]  # [S, E]
        m["eps"] = np.ascontiguousarray(
            ei.reshape(NCH, P, E).transpose(1, 0, 2)
        )
        in_maps.append(m)
    return in_maps


def _postprocess(results):
    routers, idxs, skips = [], [], []
    for r in results:
        # [P, NCH, ...] with token = ch*128 + p  -> [T, ...]
        routers.append(
            np.ascontiguousarray(r["router"].transpose(1, 0, 2)).reshape(T, E)
        )
        idxs.append(
            np.ascontiguousarray(r["idx"].transpose(1, 0, 2))
            .reshape(T, K)
            .astype(np.int32)
        )
        skips.append(np.ascontiguousarray(r["skp"].T).reshape(T, 1))
    router = np.stack(routers).reshape(B, S, E).astype(np.float32)
    indices = np.stack(idxs).reshape(B, S, K).astype(np.int32)
    skip = np.stack(skips).reshape(B, S, 1).astype(np.float32)
    return router, indices, skip


_NC_CACHE: dict[tuple, bass.Bass] = {}


def _get_nc(mode: str, bias_nonzero: bool) -> bass.Bass:
    key = (mode, bias_nonzero)
    if key not in _NC_CACHE:
        _NC_CACHE[key] = build_nc(mode, bias_nonzero)
    return _NC_CACHE[key]


def run(inputs: dict, mode: str = MATMUL_MODE, **spmd_kwargs):
    """Run on hardware; returns ((router, indices, skip), BassKernelResults)."""
    in_maps = _prep_host(mode=mode, **inputs)
    bias_nonzero = in_maps[0].pop("__bias_nonzero__")
    for m in in_maps[1:]:
        m.pop("__bias_nonzero__")
    nc = _get_nc(mode, bias_nonzero)
    res = bass_utils.run_bass_kernel_spmd(
        nc, in_maps, core_ids=list(range(NCORES)), **spmd_kwargs
    )
    return _postprocess(res.results), res


def kernel(**inputs) -> tuple:
    (router, indices, skip), _ = run(inputs)
    return router, indices, skip
